# revision 16
# baseline (speedup 1.0000x reference)
"""Trainium2 Bass kernel for nn_DSE_5428838662636 (gnn_message_passing).

Per sample: one-hot -> 3x3 conv -> relu -> 3x3 conv -> relu -> 1x1 conv
-> 4-neighbor Gaussian affinity -> 8x soft message passing (softmax over K)
-> soft pooled features.  Outputs (A, T).

Sharding: pure data parallel over batch B=32 across 8 cores (4 samples/core).
Self-contained: needs only the concourse runtime at /opt/trn_rl_repo.

Key layout choices (per core, 4 samples):
- conv stage: channels stacked on partitions for all 4 samples at once
  (block-diagonal weights), pixels on the free dim in a padded 130x130 grid
  with an extra 130-element margin so all 9 taps are plain free-dim offsets.
- conv3 is computed transposed (lhsT = y2 column) so phi lands h-major.
- message passing: h on partitions, (w, k) on free. Neighbor affinities are
  edge-symmetric -> only 2 edge fields (V vertical, H horizontal).
  Unnormalized recurrence: E' = exp(beta * Sigma_d shift_d(u_d * r * E));
  the softmax division folds into per-pixel weights r = 1/den, replicated
  across k by the scalar engine. Products run in bf16 (DVE 2x mode), the
  shift+sum runs on the tensor engine accumulating in fp32 PSUM.
- pooling: per-column matmuls accumulate A^T @ [1,h,h^2,w,w^2,G] over w.
"""
import sys

if '/opt/trn_rl_repo' not in sys.path:
    sys.path.insert(0, '/opt/trn_rl_repo')

import numpy as np
import ml_dtypes

import concourse.bass as bass
import concourse.bacc as bacc
import concourse.tile as tile
import concourse.mybir as mybir
from concourse.bass_utils import run_bass_kernel_spmd

FP32 = mybir.dt.float32
BF16 = mybir.dt.bfloat16
INT32 = mybir.dt.int32
AF = mybir.ActivationFunctionType
OP = mybir.AluOpType
AX = mybir.AxisListType

C, K, EMB = 10, 32, 32
B, H, W = 32, 128, 128
NCORES = 8
SPC = B // NCORES          # samples per core = 4
WP = W + 2                 # padded width 130
NPIX = (H + 2) * WP        # 16900 padded pixels
MARG = 2 * WP              # conv-buffer margin (two padded rows)
GLEN = NPIX + 2 * MARG     # 17420 = 134 * 130
TAU = 0.5
BETA = 2.0
NITER = 8

_CACHE = {}


# ---------------------------------------------------------------------------
# host-side constants
# ---------------------------------------------------------------------------

def _host_consts():
    if 'consts' in _CACHE:
        return _CACHE['consts']
    f32 = np.float32
    # A_init: K seeds on 4x8 grid, soft assignment (numpy copy of reference)
    gh, gw = 4, K // 4
    sh = (np.arange(gh, dtype=f32) + 0.5) * (H / gh)
    sw = (np.arange(gw, dtype=f32) + 0.5) * (W / gw)
    seed_h = np.repeat(sh, gw)
    seed_w = np.tile(sw, gh)
    hh_i = np.arange(H, dtype=f32)[:, None, None]
    ww_i = np.arange(W, dtype=f32)[None, :, None]
    d2 = (hh_i - seed_h) ** 2 + (ww_i - seed_w) ** 2   # (H, W, K)
    sigma2 = (H / gh) * (W / gw)
    z = -d2 / sigma2
    z = z - z.max(-1, keepdims=True)
    ez = np.exp(z)
    A0 = (ez / ez.sum(-1, keepdims=True)).astype(f32)  # (H, W, K)
    Ainit = np.zeros((H, WP, K), f32)
    Ainit[:, 1:W + 1, :] = A0

    # shift matrices for matmul out[m] = sum_k S[k, m] x[k]
    Sm1 = np.zeros((128, 128), f32)   # out[h] = x[h-1]
    Sm1[np.arange(127), np.arange(1, 128)] = 1.0
    Sp1 = np.zeros((128, 128), f32)   # out[h] = x[h+1]
    Sp1[np.arange(1, 128), np.arange(127)] = 1.0

    # pooling static rhs cols: Mw[h, wp, :] = [1, hh, hh^2, ww, ww^2]
    hhn = ((np.arange(H) + 0.5) / H).astype(f32)
    wwn = ((np.arange(W) + 0.5) / W).astype(f32)
    Mw = np.zeros((128, WP, 5), f32)
    Mw[:, 1:W + 1, 0] = 1.0
    Mw[:, 1:W + 1, 1] = hhn[:, None]
    Mw[:, 1:W + 1, 2] = (hhn ** 2)[:, None]
    Mw[:, 1:W + 1, 3] = wwn[None, :]
    Mw[:, 1:W + 1, 4] = (wwn ** 2)[None, :]

    ic120 = np.tile(np.arange(C, dtype=np.float32), 12)[:, None]  # (120, 1)

    out = dict(Ainit=Ainit.astype(ml_dtypes.bfloat16),
               Sm1=Sm1, Sp1=Sp1,
               Sm1_bf=Sm1.astype(ml_dtypes.bfloat16),
               Sp1_bf=Sp1.astype(ml_dtypes.bfloat16),
               I_bf=np.eye(128, dtype=ml_dtypes.bfloat16),
               Mw=Mw.reshape(128, WP * 5), ic120=ic120)
    _CACHE['consts'] = out
    return out


def _pack_weights(w1, b1, w2, b2, w3, b3):
    f32 = np.float32
    w1 = np.asarray(w1, f32); w2 = np.asarray(w2, f32); w3 = np.asarray(w3, f32)
    b1 = np.asarray(b1, f32); b2 = np.asarray(b2, f32)
    # lw1[dxi*40 + s*10 + c, dyi, s*32 + co] = w1[dyi, dxi, c, co]
    lw1 = np.zeros((120, 3, 128), f32)
    for dyi in range(3):
        for dxi in range(3):
            for s in range(SPC):
                lw1[dxi * 40 + s * 10:dxi * 40 + s * 10 + C, dyi,
                    s * 32:s * 32 + 32] = w1[dyi, dxi]
    # lw2[s*32+ci, t, s*32+co] = w2[t//3, t%3, ci, co]
    lw2 = np.zeros((128, 9, 128), f32)
    for t in range(9):
        for s in range(SPC):
            lw2[s * 32:s * 32 + 32, t, s * 32:s * 32 + 32] = w2[t // 3, t % 3]
    # block-diag pair layout for transposed conv3: rw3[p*64+q*32+ci, q*32+e]
    rw3 = np.zeros((128, 64), f32)   # b3 cancels in affinity diffs
    for p in range(2):
        for q in range(2):
            rw3[p * 64 + q * 32:p * 64 + q * 32 + 32, q * 32:q * 32 + 32] = w3[0, 0]
    bias1 = np.ascontiguousarray(np.tile(b1, SPC)[:, None])
    bias2 = np.ascontiguousarray(np.tile(b2, SPC)[:, None])
    return dict(lw1=np.ascontiguousarray(lw1.reshape(120, 3 * 128)),
                lw2=np.ascontiguousarray(lw2.reshape(128, 9 * 128)),
                rw3=rw3, bias1=bias1, bias2=bias2)


# ---------------------------------------------------------------------------
# kernel build
# ---------------------------------------------------------------------------

def _zero_ring(nc, buf):
    """Zero margins + pad ring of a [128, GLEN] padded conv buffer."""
    br = buf[:].rearrange("p (a b) -> p a b", a=GLEN // WP, b=WP)
    nc.vector.memset(br[:, 0:3, :], 0.0)        # front margin + padded row 0
    nc.vector.memset(br[:, 131:134, :], 0.0)    # padded row 129 + back margin
    nc.vector.memset(br[:, 3:131, 0:1], 0.0)    # left pad col
    nc.vector.memset(br[:, 3:131, 129:130], 0.0)  # right pad col


def _build(debug=False):
    key = ('nc', debug)
    if key in _CACHE:
        return _CACHE[key]
    nc = bacc.Bacc("TRN2", target_bir_lowering=False, debug=False)

    dX = nc.dram_tensor("X", [SPC, H, W], INT32, kind="ExternalInput")
    dlw1 = nc.dram_tensor("lw1", [120, 3 * 128], FP32, kind="ExternalInput")
    dlw2 = nc.dram_tensor("lw2", [128, 9 * 128], FP32, kind="ExternalInput")
    drw3 = nc.dram_tensor("rw3", [128, 64], FP32, kind="ExternalInput")
    dbias1 = nc.dram_tensor("bias1", [128, 1], FP32, kind="ExternalInput")
    dbias2 = nc.dram_tensor("bias2", [128, 1], FP32, kind="ExternalInput")
    dAinit = nc.dram_tensor("Ainit", [128, WP, K], BF16, kind="ExternalInput")
    dSm1 = nc.dram_tensor("Sm1", [128, 128], FP32, kind="ExternalInput")
    dSp1 = nc.dram_tensor("Sp1", [128, 128], FP32, kind="ExternalInput")
    dSm1b = nc.dram_tensor("Sm1_bf", [128, 128], BF16, kind="ExternalInput")
    dSp1b = nc.dram_tensor("Sp1_bf", [128, 128], BF16, kind="ExternalInput")
    dIb = nc.dram_tensor("I_bf", [128, 128], BF16, kind="ExternalInput")
    dMw = nc.dram_tensor("Mw", [128, WP * 5], FP32, kind="ExternalInput")
    dIC = nc.dram_tensor("ic120", [120, 1], FP32, kind="ExternalInput")

    dA = nc.dram_tensor("A_out", [SPC, H, W, K], FP32, kind="ExternalOutput")
    dT = nc.dram_tensor("T_out", [SPC, K, 17], FP32, kind="ExternalOutput")
    dbg = {}
    if debug:
        dbg['phi'] = nc.dram_tensor("phi_dbg", [SPC, 128, WP, EMB], FP32,
                                    kind="ExternalOutput")
        dbg['Vf'] = nc.dram_tensor("Vf_dbg", [SPC, 128, WP], FP32,
                                   kind="ExternalOutput")
        dbg['Hf'] = nc.dram_tensor("Hf_dbg", [SPC, 128, WP], FP32,
                                   kind="ExternalOutput")
        dbg['E1'] = nc.dram_tensor("E1_dbg", [SPC, 128, H, K], BF16,
                                   kind="ExternalOutput")

    io = dict(X=dX, lw1=dlw1, lw2=dlw2, rw3=drw3, bias1=dbias1, bias2=dbias2,
              Ainit=dAinit, Sm1=dSm1, Sp1=dSp1, Sm1_bf=dSm1b, Sp1_bf=dSp1b,
              I_bf=dIb, Mw=dMw, ic120=dIC, A_out=dA, T_out=dT)
    with tile.TileContext(nc) as tc:
        _emit(nc, tc, io, dbg)
    nc.compile()
    _CACHE[key] = nc
    return nc


def _emit(nc, tc, io, dbg):
    from contextlib import ExitStack
    with ExitStack() as ctx:
        consts = ctx.enter_context(tc.tile_pool(name="consts", bufs=1))
        small = ctx.enter_context(tc.tile_pool(name="small", bufs=1))

        def const_tile(shape, dt, name, src):
            t = consts.tile(shape, dt, tag=name, name=name)
            nc.sync.dma_start(t[:], src)
            return t

        t_b1 = const_tile([128, 1], FP32, "b1", io['bias1'][:, :])
        t_b2 = const_tile([128, 1], FP32, "b2", io['bias2'][:, :])
        t_Ainit = const_tile([128, WP, K], BF16, "Ainit",
                             io['Ainit'][:, :, :])
        t_Sp1 = const_tile([128, 128], FP32, "Sp1", io['Sp1'][:, :])
        t_Sm1b = const_tile([128, 128], BF16, "Sm1b", io['Sm1_bf'][:, :])
        t_Sp1b = const_tile([128, 128], BF16, "Sp1b", io['Sp1_bf'][:, :])
        t_Ib = const_tile([128, 128], BF16, "Ib", io['I_bf'][:, :])
        t_Mw = const_tile([128, WP, 5], FP32, "Mw",
                          io['Mw'][:, :].rearrange("p (a b) -> p a b", a=WP))
        # X in h-major for G_pool later: [128, SPC, WP] int32, pads = -1
        t_Xh = small.tile([128, SPC, WP], INT32, tag="Xh", name="Xh")
        nc.vector.memset(t_Xh[:], -1)
        for s in range(SPC):
            nc.sync.dma_start(t_Xh[:, s, 1:W + 1], io['X'][s, :, :])

        NCH = (NPIX + 511) // 512  # 34 chunks (last = 4)
        phi = []
        Vf = [small.tile([128, WP], FP32, tag=f"Vf{s}", name=f"Vf{s}") for s in range(SPC)]
        Hf = [small.tile([128, WP], FP32, tag=f"Hf{s}", name=f"Hf{s}") for s in range(SPC)]

        with tc.tile_pool(name="gbuf", bufs=1) as gbuf, \
             tc.tile_pool(name="cpsum", bufs=4, space="PSUM") as cpsum:
            g120 = gbuf.tile([120, GLEN], FP32, tag="g120", name="g120")
            # --------------------------------------------------------------
            # phase 1: stacked one-hot + conv1 -> y1
            # --------------------------------------------------------------
            with tc.tile_pool(name="y1buf", bufs=1) as y1buf, \
                 tc.tile_pool(name="cw1", bufs=1) as cw1:
                t_lw1 = cw1.tile([120, 3, 128], FP32, tag="lw1", name="lw1")
                nc.sync.dma_start(
                    t_lw1[:], io['lw1'][:, :].rearrange("p (a b) -> p a b", a=3))
                t_IC = cw1.tile([120, 1], FP32, tag="IC", name="IC")
                nc.sync.dma_start(t_IC[:], io['ic120'][:, :])

                HALF = GLEN // 2  # 8710 = 67 rows of 130
                with tc.tile_pool(name="xbuf", bufs=1) as xbuf:
                    xb = xbuf.tile([120, HALF], INT32, tag="xb", name="xb")
                    xbr = xb[:].rearrange("p (a b) -> p a b", a=67, b=WP)
                    for half in range(2):
                        f0 = half * HALF
                        h0 = half * 64
                        a0 = 3 - half * 3
                        nc.vector.memset(xb[:], -1)
                        for dxi, dx in enumerate((-1, 0, 1)):
                            for s in range(SPC):
                                p0 = dxi * 40 + s * 10
                                nc.sync.dma_start(
                                    xbr[p0:p0 + C, a0:a0 + 64,
                                        1 - dx:1 - dx + W],
                                    io['X'][s, h0:h0 + 64,
                                            :].partition_broadcast(C))
                        nc.vector.tensor_scalar(
                            g120[:, f0:f0 + HALF], xb[:], t_IC[:], None,
                            OP.is_equal)

                y1 = y1buf.tile([128, GLEN], FP32, tag="y1", name="y1")
                for ch in range(NCH):
                    q0 = ch * 512
                    n = min(512, NPIX - q0)
                    ps = cpsum.tile([128, 512], FP32, tag="cps", name="cps")
                    for dyi, dy in enumerate((-1, 0, 1)):
                        o = MARG + q0 + dy * WP
                        nc.tensor.matmul(ps[:, 0:n], t_lw1[:, dyi, :],
                                         g120[:, o:o + n],
                                         start=(dyi == 0), stop=(dyi == 2))
                    nc.scalar.activation(y1[:, MARG + q0:MARG + q0 + n],
                                         ps[:, 0:n], AF.Relu, bias=t_b1[:],
                                         scale=1.0)
                _zero_ring(nc, y1)

                # ----------------------------------------------------------
                # phase 2: conv2 -> y2 (reuses g120's slot)
                # ----------------------------------------------------------
                y2 = gbuf.tile([128, GLEN], FP32, tag="g120", name="g120")
                with tc.tile_pool(name="cw2", bufs=1) as cw2:
                    t_lw2 = cw2.tile([128, 9, 128], FP32, tag="lw2", name="lw2")
                    nc.sync.dma_start(
                        t_lw2[:],
                        io['lw2'][:, :].rearrange("p (a b) -> p a b", a=9))
                    for ch in range(NCH):
                        q0 = ch * 512
                        n = min(512, NPIX - q0)
                        ps = cpsum.tile([128, 512], FP32, tag="cps", name="cps")
                        for t in range(9):
                            o = MARG + q0 + (t // 3 - 1) * WP + (t % 3 - 1)
                            nc.tensor.matmul(ps[:, 0:n], t_lw2[:, t, :],
                                             y1[:, o:o + n],
                                             start=(t == 0), stop=(t == 8))
                        nc.scalar.activation(y2[:, MARG + q0:MARG + q0 + n],
                                             ps[:, 0:n], AF.Relu, bias=t_b2[:],
                                             scale=1.0)
                    _zero_ring(nc, y2)

            # --------------------------------------------------------------
            # phase 3+4: conv3 transposed -> phi; affinity fields Vf, Hf
            # (phi pool closes before message passing to free SBUF)
            # --------------------------------------------------------------
            with tc.tile_pool(name="phi", bufs=1) as phipool:
                t_rw3 = phipool.tile([128, 64], FP32, tag="rw3", name="rw3")
                nc.sync.dma_start(t_rw3[:], io['rw3'][:, :])
                t_Sm1 = phipool.tile([128, 128], FP32, tag="Sm1", name="Sm1")
                nc.sync.dma_start(t_Sm1[:], io['Sm1'][:, :])
                y2r = y2[:].rearrange("p (a b) -> p a b", a=GLEN // WP, b=WP)
                for s in range(SPC):
                    p = phipool.tile([128, WP, EMB], FP32, tag=f"phi{s}", name=f"phi{s}")
                    phi.append(p)
                    nc.vector.memset(p[:], 0.0)
                for pr in range(2):        # sample pairs (0,1) and (2,3)
                    for g in range(16):    # col groups of 8, wp in [1, 129)
                        wp0 = 1 + g * 8
                        ps3 = cpsum.tile([128, 8, 64], FP32, tag="cps", name="cps")
                        for j in range(8):
                            lhsT = y2r[pr * 64:pr * 64 + 64, 3:131, wp0 + j]
                            nc.tensor.matmul(ps3[:, j, :], lhsT,
                                             t_rw3[pr * 64:pr * 64 + 64, :],
                                             start=True, stop=True)
                        for q in range(2):
                            nc.scalar.activation(
                                phi[pr * 2 + q][:, wp0:wp0 + 8, :],
                                ps3[:, :, q * 32:q * 32 + 32],
                                AF.Copy, bias=0.0, scale=1.0)
                if dbg:
                    for s in range(SPC):
                        nc.sync.dma_start(dbg['phi'][s, :, :, :], phi[s][:])

                with tc.tile_pool(name="affbuf", bufs=1) as affbuf:
                    for s in range(SPC):
                        v, h = Vf[s], Hf[s]
                        phs = phi[s][:].rearrange("p a b -> p (a b)")
                        dV = affbuf.tile([128, WP * EMB], FP32, tag="dV", name="dV")
                        for ch in range(9):
                            q0 = ch * 512
                            n = min(512, WP * EMB - q0)
                            ps = cpsum.tile([128, 512], FP32, tag="cps", name="cps")
                            nc.tensor.matmul(ps[:, 0:n], t_Sm1[:],
                                             phs[:, q0:q0 + n],
                                             start=True, stop=True)
                            nc.vector.tensor_tensor(dV[:, q0:q0 + n],
                                                    phs[:, q0:q0 + n],
                                                    ps[:, 0:n], OP.subtract)
                        sq = affbuf.tile([128, WP, EMB], FP32, tag="sq", name="sq")
                        dVr = dV[:].rearrange("p (a b) -> p a b", a=WP, b=EMB)
                        nc.scalar.activation(sq[:], dVr, AF.Square, bias=0.0,
                                             scale=1.0)
                        nc.vector.tensor_reduce(v[:], sq[:], AX.X, OP.add)
                        nc.scalar.activation(v[:], v[:], AF.Exp, bias=0.0,
                                             scale=-1.0 / TAU)
                        nc.vector.memset(v[0:1, :], 0.0)  # no edge above row 0
                        dH = affbuf.tile([128, WP, EMB], FP32, tag="dV", name="dV")
                        nc.vector.tensor_tensor(dH[:, 1:WP, :],
                                                phi[s][:, 1:WP, :],
                                                phi[s][:, 0:WP - 1, :],
                                                OP.subtract)
                        sqh = affbuf.tile([128, WP, EMB], FP32, tag="sq", name="sq")
                        nc.scalar.activation(sqh[:, 1:WP, :], dH[:, 1:WP, :],
                                             AF.Square, bias=0.0, scale=1.0)
                        nc.vector.tensor_reduce(h[:, 1:WP], sqh[:, 1:WP, :],
                                                AX.X, OP.add)
                        nc.scalar.activation(h[:, 1:WP], h[:, 1:WP], AF.Exp,
                                             bias=0.0, scale=-1.0 / TAU)
                        nc.vector.memset(h[:, 0:2], 0.0)        # wp 0,1
                        nc.vector.memset(h[:, WP - 1:WP], 0.0)  # wp 129
                        if dbg:
                            nc.sync.dma_start(dbg['Vf'][s, :, :], v[:])
                            nc.sync.dma_start(dbg['Hf'][s, :, :], h[:])

        # ------------------------------------------------------------------
        # phase 5+6: message passing (2 samples at a time) + finalize
        # ------------------------------------------------------------------
        mp = ctx.enter_context(tc.tile_pool(name="mp", bufs=1))
        urep_pool = ctx.enter_context(tc.tile_pool(name="urep", bufs=2))
        rrep_pool = ctx.enter_context(tc.tile_pool(name="rrep", bufs=1))
        tpool = ctx.enter_context(tc.tile_pool(name="tprod", bufs=1))
        mpsum = ctx.enter_context(tc.tile_pool(name="mpsum", bufs=2,
                                               space="PSUM"))
        spsum = ctx.enter_context(tc.tile_pool(name="spsum", bufs=1,
                                               space="PSUM"))
        gpool = ctx.enter_context(tc.tile_pool(name="gpool", bufs=1))
        apool = ctx.enter_context(tc.tile_pool(name="apool", bufs=1))
        ppsum = ctx.enter_context(tc.tile_pool(name="ppsum", bufs=2,
                                               space="PSUM"))
        tsb = ctx.enter_context(tc.tile_pool(name="tsb", bufs=2))

        E = [mp.tile([128, H, K], BF16, tag=f"E{i}", name=f"E{i}") for i in range(2)]
        E8f = mp.tile([128, H, K], FP32, tag="E8f", name="E8f")
        Ftile = [mp.tile([128, WP, K], BF16, tag=f"F{i}", name=f"F{i}") for i in range(2)]
        for i in range(2):
            nc.vector.memset(Ftile[i][:, 0:1, :], 0.0)
            nc.vector.memset(Ftile[i][:, WP - 1:WP, :], 0.0)
        den = [small.tile([128, H], FP32, tag=f"den{i}", name=f"den{i}") for i in range(2)]
        rr = [small.tile([128, H], FP32, tag=f"r{i}", name=f"r{i}") for i in range(2)]
        # products: up/dn/lt are consumed unshifted in w, rt shifted by +1
        T_up = tpool.tile([128, H, K], BF16, tag="Tup", name="Tup")
        T_dn = tpool.tile([128, H, K], BF16, tag="Tdn", name="Tdn")
        T_lt = tpool.tile([128, H, K], BF16, tag="Tlt", name="Tlt")
        T_rt = tpool.tile([128, WP, K], BF16, tag="Trt", name="Trt")
        nc.vector.memset(T_rt[:], 0.0)

        def mp_iter(s, it, ur):
            i = s % 2
            last = it == NITER - 1
            if it == 0:
                Fi = t_Ainit[:, 1:W + 1, :]      # normalized init, den = 1
                Fm = t_Ainit[:, 0:W, :]          # w-shifted read
            else:
                nc.vector.tensor_reduce(den[i][:], E[i][:], AX.X, OP.add)
                nc.vector.reciprocal(rr[i][:], den[i][:])
                rrep = rrep_pool.tile([128, H, K], BF16, tag="rrep", name="rrep")
                nc.scalar.activation(rrep[:],
                                     rr[i][:].to_broadcast((128, H, K)),
                                     AF.Copy, bias=0.0, scale=1.0)
                F = Ftile[i]
                nc.vector.tensor_tensor(F[:, 1:W + 1, :], E[i][:], rrep[:],
                                        OP.mult)
                Fi = F[:, 1:W + 1, :]
                Fm = F[:, 0:W, :]
            # products (bf16, 2x DVE): T_lt[wp] = Hf[wp] * F[wp-1]
            nc.vector.tensor_tensor(T_up[:], ur[0][:], Fi, OP.mult)
            nc.vector.tensor_tensor(T_dn[:], ur[1][:], Fi, OP.mult)
            nc.vector.tensor_tensor(T_lt[:], ur[2][:], Fm, OP.mult)
            nc.vector.tensor_tensor(T_rt[:, 1:W + 1, :], ur[2][:], Fi, OP.mult)
            # agg = shift-sum on PE; exp on ACT
            for cnk in range(4):
                w0 = 1 + cnk * 32                # padded-w of chunk start
                ps = mpsum.tile([128, 32, K], FP32, tag="mps", name="mps")
                for hlf in range(2):
                    wi = w0 - 1 + hlf * 16       # interior col index
                    po = ps[:, hlf * 16:hlf * 16 + 16, :]
                    nc.tensor.matmul(po, t_Sm1b[:], T_up[:, wi:wi + 16, :],
                                     start=True, stop=False)
                    nc.tensor.matmul(po, t_Sp1b[:], T_dn[:, wi:wi + 16, :],
                                     start=False, stop=False)
                    nc.tensor.matmul(po, t_Ib[:], T_lt[:, wi:wi + 16, :],
                                     start=False, stop=False)
                    nc.tensor.matmul(po, t_Ib[:],
                                     T_rt[:, wi + 2:wi + 18, :],
                                     start=False, stop=True)
                dst = E8f if last else E[i]
                nc.scalar.activation(dst[:, w0 - 1:w0 + 31, :], ps[:],
                                     AF.Exp, bias=0.0, scale=BETA)
            if it == 0 and dbg:
                nc.sync.dma_start(dbg['E1'][s, :, :, :], E[i][:])

        def build_urep(s):
            # u_up[h] = Vf[h+1], u_dn[h] = Vf[h], u_rt[wp] = Hf[wp]
            ps = spsum.tile([128, WP], FP32, tag="vps", name="vps")
            nc.tensor.matmul(ps[:], t_Sp1[:], Vf[s][:], start=True, stop=True)
            ur = [urep_pool.tile([128, H, K], BF16, tag=f"u{d}", name=f"u{d}")
                  for d in range(3)]
            nc.scalar.activation(ur[0][:],
                                 ps[:, 1:W + 1].to_broadcast((128, H, K)),
                                 AF.Copy, bias=0.0, scale=1.0)
            nc.scalar.activation(ur[1][:],
                                 Vf[s][:, 1:W + 1].to_broadcast((128, H, K)),
                                 AF.Copy, bias=0.0, scale=1.0)
            nc.scalar.activation(ur[2][:],
                                 Hf[s][:, 1:W + 1].to_broadcast((128, H, K)),
                                 AF.Copy, bias=0.0, scale=1.0)
            return ur

        def finalize(s):
            i = s % 2
            nc.vector.tensor_reduce(den[i][:], E8f[:], AX.X, OP.add)
            nc.vector.reciprocal(rr[i][:], den[i][:])
            A_sb = apool.tile([128, H, K], FP32, tag="Asb", name="Asb")
            nc.vector.tensor_tensor(A_sb[:], E8f[:],
                                    rr[i][:].to_broadcast((128, H, K)), OP.mult)
            nc.sync.dma_start(io['A_out'][s, :, :, :], A_sb[:])

            Gp = gpool.tile([128, WP, 15], FP32, tag="Gp", name="Gp")
            nc.vector.tensor_copy(Gp[:, :, 0:5], t_Mw[:])
            xf = small.tile([128, WP], FP32, tag="xf", name="xf")
            nc.vector.tensor_copy(xf[:], t_Xh[:, s, :])
            for c in range(C):
                nc.vector.tensor_scalar(Gp[:, :, 5 + c], xf[:], float(c), None,
                                        OP.is_equal)
            pT = ppsum.tile([32, 15], FP32, tag="pT", name="pT")
            for w in range(W):
                nc.tensor.matmul(pT[:], A_sb[:, w, :], Gp[:, w + 1, :],
                                 start=(w == 0), stop=(w == W - 1))
            # features: pT cols = [mass, hm, h2m, wm, w2m, col0..9]
            Traw = tsb.tile([32, 15], FP32, tag="Traw", name="Traw")
            mass = tsb.tile([32, 1], FP32, tag="mass", name="mass")
            rm = tsb.tile([32, 1], FP32, tag="rm", name="rm")
            Tt = tsb.tile([32, 17], FP32, tag="Tt", name="Tt")
            nc.vector.tensor_scalar(mass[:], pT[:, 0:1], 1e-6, None, OP.add)
            nc.vector.reciprocal(rm[:], mass[:])
            nc.vector.tensor_scalar(Traw[:, 1:15], pT[:, 1:15], rm[:], None,
                                    OP.mult)
            nc.vector.tensor_scalar(Tt[:, 0:1], mass[:], 1.0 / (H * W), None,
                                    OP.mult)
            nc.vector.tensor_copy(Tt[:, 1:2], Traw[:, 1:2])    # h_c
            nc.vector.tensor_copy(Tt[:, 2:3], Traw[:, 3:4])    # w_c
            nc.vector.tensor_copy(Tt[:, 3:13], Traw[:, 5:15])  # col
            cen = tsb.tile([32, 2], FP32, tag="cen", name="cen")
            nc.vector.tensor_copy(cen[:, 0:1], Traw[:, 1:2])
            nc.vector.tensor_copy(cen[:, 1:2], Traw[:, 3:4])
            m2 = tsb.tile([32, 2], FP32, tag="m2", name="m2")
            nc.vector.tensor_copy(m2[:, 0:1], Traw[:, 2:3])
            nc.vector.tensor_copy(m2[:, 1:2], Traw[:, 4:5])
            sq2 = tsb.tile([32, 2], FP32, tag="sq2", name="sq2")
            nc.vector.tensor_tensor(sq2[:], cen[:], cen[:], OP.mult)
            nc.vector.tensor_tensor(m2[:], m2[:], sq2[:], OP.subtract)
            nc.vector.tensor_scalar(m2[:], m2[:], 0.0, None, OP.max)
            nc.vector.tensor_scalar(m2[:], m2[:], 1e-6, None, OP.add)
            sd = tsb.tile([32, 2], FP32, tag="sd", name="sd")
            nc.scalar.activation(sd[:], m2[:], AF.Sqrt, bias=0.0, scale=1.0)
            nc.vector.tensor_tensor(Tt[:, 13:14], cen[:, 0:1], sd[:, 0:1],
                                    OP.subtract)
            nc.vector.tensor_tensor(Tt[:, 14:15], cen[:, 0:1], sd[:, 0:1],
                                    OP.add)
            nc.vector.tensor_tensor(Tt[:, 15:16], cen[:, 1:2], sd[:, 1:2],
                                    OP.subtract)
            nc.vector.tensor_tensor(Tt[:, 16:17], cen[:, 1:2], sd[:, 1:2],
                                    OP.add)
            nc.sync.dma_start(io['T_out'][s, :, :], Tt[:])

        for grp in range(2):
            ss = (2 * grp, 2 * grp + 1)
            urs = {s: build_urep(s) for s in ss}
            for it in range(NITER - 1):
                for s in ss:
                    mp_iter(s, it, urs[s])
            for s in ss:
                mp_iter(s, NITER - 1, urs[s])
                finalize(s)


# ---------------------------------------------------------------------------
# public entry
# ---------------------------------------------------------------------------

def _in_maps(X, w1, b1, w2, b2, w3, b3):
    cst = _host_consts()
    wts = _pack_weights(w1, b1, w2, b2, w3, b3)
    X = np.asarray(X, np.int32)
    shared = dict(lw1=wts['lw1'], lw2=wts['lw2'], rw3=wts['rw3'],
                  bias1=wts['bias1'], bias2=wts['bias2'],
                  Ainit=np.ascontiguousarray(cst['Ainit']),
                  Sm1=cst['Sm1'], Sp1=cst['Sp1'], Sm1_bf=cst['Sm1_bf'],
                  Sp1_bf=cst['Sp1_bf'], I_bf=cst['I_bf'],
                  Mw=np.ascontiguousarray(cst['Mw']), ic120=cst['ic120'])
    return [dict(shared, X=np.ascontiguousarray(X[c * SPC:(c + 1) * SPC]))
            for c in range(NCORES)]


def kernel(X, w1, b1, w2, b2, w3, b3):
    nc = _build(debug=False)
    maps = _in_maps(X, w1, b1, w2, b2, w3, b3)
    res = run_bass_kernel_spmd(nc, maps, list(range(NCORES)))
    A = np.concatenate([res.results[c]['A_out'] for c in range(NCORES)], axis=0)
    T = np.concatenate([res.results[c]['T_out'] for c in range(NCORES)], axis=0)
    return A, T
